# revision 3
# baseline (speedup 1.0000x reference)
"""Trainium2 Bass kernel for BSplineBasis (KAN-style cubic B-spline basis).

Math: reference computes Cox-de Boor recursion (order 3) over a uniform grid
(grid_size=5, order=3, range [-0.5, 1.5], h=0.4).  For x in [0,1) only cells
j in {4,5,6} occur, and the basis row has exactly 4 non-zeros out of 8:
    out[n, 8f + (j-3..j)] = v0..v3,  t = (x - g_j)/h
    v0=(1-t)^3/6, v1=(3t^3-6t^2+4)/6, v2=v1(1-t), v3=t^3/6
Kernel strategy per [128, 512] x-tile:
  - masks  m4=[x<g5], m6=[x>=g6];  t = (x-g4)/h - 1 + m4 - m6
  - write default cell j=5 pattern (v0..v3 at slots 2..5) into the
    interleaved [128, 4096] out tile; slots 0,7 pre-zeroed once per buffer
  - slot1 = m4*v0, slot6 = m6*v3
  - two copy_predicated shift chains re-place the 4 values for j=6 / j=4
Sharding: pure data-parallel over batch across 8 cores (4096 rows each).
"""

import sys

sys.path.insert(0, "/opt/trn_rl_repo")

import numpy as np

import concourse.bacc as bacc
import concourse.tile as tile
from concourse import mybir
from concourse.bass_utils import run_bass_kernel_spmd

N_CORES = 8
P = 128
F = 512  # in_features
E = 8    # basis values per feature (grid_size + spline_order)

AF = mybir.ActivationFunctionType
OP = mybir.AluOpType

_PROGRAM_CACHE: dict = {}


def _build_program(rows: int, consts: tuple):
    inv_h, u_bias, g5, g6 = consts
    nc = bacc.Bacc("TRN2", target_bir_lowering=False, debug=False,
                   num_devices=N_CORES)
    f32 = mybir.dt.float32
    x = nc.declare_dram_parameter("x", [rows, F], f32, isOutput=False)
    out = nc.declare_dram_parameter("out", [rows, F * E], f32, isOutput=True)
    ntiles = rows // P
    OUT_BUFS = 3

    with tile.TileContext(nc) as tc:
        with (
            tc.tile_pool(name="io", bufs=3) as io,
            tc.tile_pool(name="mid", bufs=2) as mid,
            tc.tile_pool(name="outp", bufs=OUT_BUFS) as outp,
            tc.tile_pool(name="zpool", bufs=1) as zpool,
        ):
            zero = zpool.tile([P, F], f32, tag="zero")
            nc.vector.memset(zero, 0.0)
            # Pre-zero every out slot once; slots 0 and 7 of each 8-group are
            # never written in the loop, so they stay zero across iterations.
            for _ in range(OUT_BUFS):
                ot0 = outp.tile([P, F * E], f32, tag="out")
                nc.gpsimd.memset(ot0, 0.0)

            for i in range(ntiles):
                xt = io.tile([P, F], f32, tag="x")
                nc.sync.dma_start(out=xt, in_=x[i * P:(i + 1) * P, :])

                ot = outp.tile([P, F * E], f32, tag="out")
                og = ot.rearrange("p (f e) -> p f e", e=E)

                u4p = mid.tile([P, F], f32, tag="u4p")
                nc.scalar.activation(u4p, xt, AF.Copy, bias=u_bias, scale=inv_h)
                m4 = mid.tile([P, F], f32, tag="m4")
                nc.vector.tensor_scalar(m4, xt, g5, None, OP.is_lt)
                m6 = mid.tile([P, F], f32, tag="m6")
                nc.vector.tensor_scalar(m6, xt, g6, None, OP.is_ge)
                # uint8 mask copies for copy_predicated (walrus requires int mask)
                m4u = mid.tile([P, F], mybir.dt.uint8, tag="m4u")
                nc.gpsimd.tensor_copy(m4u, m4)
                m6u = mid.tile([P, F], mybir.dt.uint8, tag="m6u")
                nc.gpsimd.tensor_copy(m6u, m6)
                tA = mid.tile([P, F], f32, tag="tA")
                nc.vector.tensor_tensor(tA, u4p, m4, OP.add)
                t = mid.tile([P, F], f32, tag="t")
                nc.vector.tensor_tensor(t, tA, m6, OP.subtract)

                w = mid.tile([P, F], f32, tag="w")
                nc.scalar.activation(w, t, AF.Copy, bias=1.0, scale=-1.0)
                t2 = mid.tile([P, F], f32, tag="t2")
                nc.scalar.activation(t2, t, AF.Square)
                w2 = mid.tile([P, F], f32, tag="w2")
                nc.scalar.activation(w2, w, AF.Square)
                qt = mid.tile([P, F], f32, tag="qt")
                nc.scalar.activation(qt, t, AF.Copy, bias=-1.0, scale=0.5)
                qw = mid.tile([P, F], f32, tag="qw")
                nc.scalar.activation(qw, w, AF.Copy, bias=-1.0, scale=0.5)
                t6 = mid.tile([P, F], f32, tag="t6")
                nc.scalar.activation(t6, t, AF.Copy, scale=1.0 / 6.0)
                w6 = mid.tile([P, F], f32, tag="w6")
                nc.scalar.activation(w6, w, AF.Copy, scale=1.0 / 6.0)

                # default cell j=5: slots 2..5 = v0..v3
                nc.vector.tensor_tensor(og[:, :, 5], t2, t6, OP.mult)  # v3
                nc.vector.tensor_tensor(og[:, :, 2], w2, w6, OP.mult)  # v0
                rt = mid.tile([P, F], f32, tag="rt")
                nc.gpsimd.tensor_tensor(rt, qt, t2, OP.mult)
                rw = mid.tile([P, F], f32, tag="rw")
                nc.gpsimd.tensor_tensor(rw, qw, w2, OP.mult)
                nc.scalar.activation(og[:, :, 3], rt, AF.Copy, bias=2.0 / 3.0)  # v1
                nc.scalar.activation(og[:, :, 4], rw, AF.Copy, bias=2.0 / 3.0)  # v2

                # edge slots
                nc.gpsimd.tensor_tensor(og[:, :, 1], m4, og[:, :, 2], OP.mult)
                nc.gpsimd.tensor_tensor(og[:, :, 6], m6, og[:, :, 5], OP.mult)

                cp = nc.vector.copy_predicated
                # j=6: shift slots 2..5 right by one (backward order)
                cp(og[:, :, 5], m6u, og[:, :, 4])
                cp(og[:, :, 4], m6u, og[:, :, 3])
                cp(og[:, :, 3], m6u, og[:, :, 2])
                cp(og[:, :, 2], m6u, zero)
                # j=4: shift slots 2..5 left by one (forward order)
                cp(og[:, :, 2], m4u, og[:, :, 3])
                cp(og[:, :, 3], m4u, og[:, :, 4])
                cp(og[:, :, 4], m4u, og[:, :, 5])
                cp(og[:, :, 5], m4u, zero)

                nc.sync.dma_start(out=out[i * P:(i + 1) * P, :], in_=ot)

    nc.compile()
    return nc


def _get_program(rows: int, consts: tuple):
    key = (rows, consts)
    if key not in _PROGRAM_CACHE:
        _PROGRAM_CACHE[key] = _build_program(rows, consts)
    return _PROGRAM_CACHE[key]


def kernel(x, grid):
    x = np.ascontiguousarray(np.asarray(x, dtype=np.float32))
    grid = np.asarray(grid, dtype=np.float32)
    n, f = x.shape
    assert f == F and n % (N_CORES * P) == 0, (n, f)
    rows = n // N_CORES

    g4 = np.float32(grid[0, 4])
    g5 = np.float32(grid[0, 5])
    g6 = np.float32(grid[0, 6])
    h = np.float32(grid[0, 5] - grid[0, 4])
    inv_h = np.float32(np.float32(1.0) / h)
    # u4p = (x - g4)/h - 1 evaluated as fma(x, inv_h, u_bias)
    u_bias = np.float32(-np.float64(g4) * np.float64(inv_h) - 1.0)

    consts = (float(inv_h), float(u_bias), float(g5), float(g6))
    nc = _get_program(rows, consts)
    in_maps = [{"x": x[c * rows:(c + 1) * rows]} for c in range(N_CORES)]
    res = run_bass_kernel_spmd(nc, in_maps, list(range(N_CORES)))
    return np.concatenate([res.results[c]["out"] for c in range(N_CORES)], axis=0)
